# revision 8
# baseline (speedup 1.0000x reference)
"""GQA causal attention (B=1, S=2048, D=4096, H=32, KV=8) on 8 trn2 cores.

Strategy: tensor-parallel over heads. Core i owns q-heads 4i..4i+3 and
kv-head i. Host pre-transposes weights/x so every matmul contracts along
the partition dim, and pre-permutes wq/wk rows (even|odd interleave ->
[evens;odds]) so RoPE becomes partition-aligned elementwise math.
Attention is computed head-locally in a scores^T [t, s] layout; softmax
normalization sums come from a ones-vector matmul. After attention an
AllGather collects all 32 heads' outputs (bf16); each core then computes
its 512-column shard of the output projection. Host concatenates shards.

Matmul operands are bf16; accumulation, softmax and RoPE math are fp32.
"""

import sys

import numpy as np

sys.path.insert(0, "/opt/trn_rl_repo")

import ml_dtypes  # noqa: E402

import concourse.bass as bass  # noqa: E402
from concourse import bacc  # noqa: E402
import concourse.mybir as mybir  # noqa: E402
import concourse.tile as tile  # noqa: E402
from concourse.bass_utils import run_bass_kernel_spmd  # noqa: E402

F32 = mybir.dt.float32
BF16 = mybir.dt.bfloat16
NPBF = ml_dtypes.bfloat16

B, S, D = 1, 2048, 4096
H, KV, HD = 32, 8, 128
NCORES = 8
HPC = H // NCORES  # q heads per core = 4
EQ = HPC * HD  # 512 local q features
NE = HPC + 2  # e-tiles per core: 4 q + 1 k + 1 v
SB = 512  # attention s block
NSB = S // SB  # 4
XSB = 512  # phase-1 s sub-block (matmul moving dim)
NXSB = S // XSB  # 4
DO = D // 128  # 32 contraction tiles for projections
EO = (H * HD) // 128  # 32 contraction tiles for out-proj
TT = S // 128  # 16 t-tiles
RG = [list(range(NCORES))]


def build():
    nc = bacc.Bacc("TRN2", target_bir_lowering=False)
    xt = nc.dram_tensor("xt", [D, S], BF16, kind="ExternalInput")
    wqkvt = nc.dram_tensor("wqkvt", [D, NE * 128], BF16, kind="ExternalInput")
    wot = nc.dram_tensor("wot", [D, EQ], BF16, kind="ExternalInput")
    cc = nc.dram_tensor("cc", [128, S], F32, kind="ExternalInput")
    ss = nc.dram_tensor("ss", [128, S], F32, kind="ExternalInput")
    masks = nc.dram_tensor("masks", [HPC, 128, SB], BF16,
                           kind="ExternalInput")
    ones = nc.dram_tensor("ones", [128, 1], BF16, kind="ExternalInput")
    ident = nc.dram_tensor("ident", [128, 128], BF16, kind="ExternalInput")
    out = nc.dram_tensor("out", [S, EQ], F32, kind="ExternalOutput")

    xt_t = xt[:].rearrange("(do p) s -> p do s", p=128)
    w_t = wqkvt[:].rearrange("(do p) e -> p do e", p=128)
    wo_t = wot[:].rearrange("(eo p) d -> p eo d", p=128)

    with tile.TileContext(nc) as tc:
        with tc.tile_pool(name="dram", bufs=1, space="DRAM") as dram:
            qkv_d = dram.tile([NE * 128, S], BF16)
            ag_in = dram.tile([EQ, S], BF16)
            ag_out = dram.tile([H * HD, S], BF16, addr_space="Shared")
            qkv_d_t = qkv_d.rearrange("(e p) s -> p e s", p=128)
            ag_in_t = ag_in.rearrange("(h p) s -> p h s", p=128)
            ag_out_t = ag_out.rearrange("(eo p) s -> p eo s", p=128)

            # ---------------- Phase 1: fused QKV projection + RoPE ----------
            with tc.tile_pool(name="p1w", bufs=1) as p1w, \
                    tc.tile_pool(name="p1x", bufs=2) as p1x, \
                    tc.tile_pool(name="p1t", bufs=2) as p1t, \
                    tc.tile_pool(name="p1ps", bufs=3, space="PSUM") as p1ps:
                w = p1w.tile([128, DO, NE * 128], BF16)
                nc.sync.dma_start(w, w_t)
                for sb in range(NXSB):
                    ssl = slice(sb * XSB, (sb + 1) * XSB)
                    xtile = p1x.tile([128, DO, XSB], BF16, tag="x")
                    nc.sync.dma_start(xtile, xt_t[:, :, ssl])
                    cct = p1x.tile([128, XSB], F32, tag="cc")
                    sst = p1x.tile([128, XSB], F32, tag="ss")
                    nc.sync.dma_start(cct, cc[:][:, ssl])
                    nc.sync.dma_start(sst, ss[:][:, ssl])
                    outsb = p1t.tile([128, NE, XSB], BF16, tag="osb")
                    atile = p1t.tile([128, HPC + 1, XSB], F32, tag="at", bufs=1)
                    btile = p1t.tile([128, HPC + 1, XSB], F32, tag="bt", bufs=1)
                    bsw = p1t.tile([128, (HPC + 1) * XSB], F32, tag="bsw", bufs=1)
                    for et in range(NE):
                        ps = p1ps.tile([128, XSB], F32, tag="ps")
                        for do in range(DO):
                            nc.tensor.matmul(
                                ps,
                                w[:, do, et * 128:(et + 1) * 128],
                                xtile[:, do, :],
                                start=(do == 0),
                                stop=(do == DO - 1),
                            )
                        if et < HPC + 1:
                            # t*cos and t*sin halves; swap+add finishes RoPE
                            nc.vector.tensor_mul(atile[:, et, :], ps, cct)
                            nc.vector.tensor_mul(btile[:, et, :], ps, sst)
                        else:
                            nc.any.tensor_copy(outsb[:, et, :], ps)
                    bt2 = btile.rearrange("p e s -> p (e s)")
                    nc.sync.dma_start(bsw[:64, :], bt2[64:, :])
                    nc.sync.dma_start(bsw[64:, :], bt2[:64, :])
                    at2 = atile.rearrange("p e s -> p (e s)")
                    ob2 = outsb.rearrange("p e s -> p (e s)")
                    nc.vector.tensor_add(
                        ob2[:, : (HPC + 1) * XSB], at2, bsw)
                    nc.sync.dma_start(qkv_d_t[:, :, ssl], outsb)

            # ---------------- Phase 2: head-local causal attention ----------
            with tc.tile_pool(name="p3w", bufs=1) as p3w:
                w3 = p3w.tile([128, EO, EQ], BF16)
                nc.sync.dma_start(w3, wo_t)

                with tc.tile_pool(name="p2qkv", bufs=1) as p2qkv, \
                        tc.tile_pool(name="p2c", bufs=1) as p2c, \
                        tc.tile_pool(name="p2e", bufs=3) as p2e, \
                        tc.tile_pool(name="p2t", bufs=2) as p2t, \
                        tc.tile_pool(name="psS", bufs=2, space="PSUM") as psS, \
                        tc.tile_pool(name="psA", bufs=2, space="PSUM") as psA, \
                        tc.tile_pool(name="psN", bufs=2, space="PSUM") as psN:
                    qe = []
                    for et in range(NE):
                        t = p2qkv.tile([128, S], BF16, name=f"qe{et}",
                                       tag=f"qe{et}")
                        nc.sync.dma_start(t, qkv_d_t[:, et, :])
                        qe.append(t)
                    mt = p2c.tile([128, HPC, SB], BF16)
                    nc.sync.dma_start(mt, masks[:].rearrange("m p s -> p m s"))
                    on = p2c.tile([128, 1], BF16)
                    nc.sync.dma_start(on, ones[:])
                    idt = p2c.tile([128, 128], BF16)
                    nc.sync.dma_start(idt, ident[:])

                    # v from [hd, s] e-tile layout to natural [t, hd] tiles
                    vn = p2c.tile([128, TT, HD], BF16)
                    for tt in range(TT):
                        pst = psS.tile([128, 128], BF16, tag="tp")
                        nc.tensor.transpose(
                            pst, qe[HPC + 1][:, tt * 128:(tt + 1) * 128], idt)
                        nc.any.tensor_copy(vn[:, tt, :], pst)

                    for h in range(HPC):
                        for b in range(NSB):
                            bsl = slice(b * SB, (b + 1) * SB)
                            pa = psA.tile([128, SB], F32, tag="av")
                            pn = psN.tile([1, SB], F32, tag="nrm")
                            ntt = 4 * b + 4
                            for j in range(ntt):
                                psc = psS.tile([128, SB], F32, tag="sc")
                                nc.tensor.matmul(
                                    psc,
                                    qe[HPC][:, j * 128:(j + 1) * 128],
                                    qe[h][:, bsl],
                                    start=True, stop=True)
                                ex = p2e.tile([128, SB], BF16, tag="ex")
                                nc.scalar.activation(
                                    ex, psc, mybir.ActivationFunctionType.Exp)
                                if j >= 4 * b:
                                    nc.vector.tensor_mul(
                                        ex, ex, mt[:, j - 4 * b, :])
                                nc.tensor.matmul(
                                    pa, vn[:, j, :], ex,
                                    start=(j == 0), stop=(j == ntt - 1))
                                nc.tensor.matmul(
                                    pn[:], on, ex,
                                    start=(j == 0), stop=(j == ntt - 1))
                            rc = p2t.tile([1, SB], F32, tag="rc")
                            nc.vector.reciprocal(rc, pn)
                            rb = p2t.tile([128, SB], F32, tag="rb")
                            nc.gpsimd.partition_broadcast(rb, rc)
                            ao = p2t.tile([128, SB], BF16, tag="ao")
                            nc.vector.tensor_mul(ao, pa, rb)
                            nc.sync.dma_start(ag_in_t[:, h, bsl], ao)

                    nc.gpsimd.collective_compute(
                        "AllGather",
                        mybir.AluOpType.bypass,
                        ins=[ag_in.opt()],
                        outs=[ag_out.opt()],
                        replica_groups=RG,
                    )

                # ---------------- Phase 3: output projection shard ----------
                with tc.tile_pool(name="p3a", bufs=2) as p3a, \
                        tc.tile_pool(name="p3o", bufs=2) as p3o, \
                        tc.tile_pool(name="ps3", bufs=2, space="PSUM") as ps3:
                    for st in range(TT):
                        stl = slice(st * 128, (st + 1) * 128)
                        at = p3a.tile([128, EO, 128], BF16, tag="aot")
                        nc.sync.dma_start(at, ag_out_t[:, :, stl])
                        po = ps3.tile([128, EQ], F32, tag="wo")
                        for eo in range(EO):
                            nc.tensor.matmul(
                                po, at[:, eo, :], w3[:, eo, :],
                                start=(eo == 0), stop=(eo == EO - 1))
                        ot = p3o.tile([128, EQ], F32, tag="ot")
                        nc.any.tensor_copy(ot, po)
                        nc.sync.dma_start(out[:][stl, :], ot)
    nc.compile()
    return nc


_CACHE = {}


def _get_program():
    if "nc" not in _CACHE:
        _CACHE["nc"] = build()
    return _CACHE["nc"]


def _host_prep(x, freqs_cos, freqs_sin, wq, wk, wv, wo):
    x2 = np.ascontiguousarray(np.asarray(x, np.float32).reshape(S, D))
    xT = np.ascontiguousarray(x2.T).astype(NPBF)
    # even|odd -> [evens;odds] row permutation per head (RoPE partition split)
    perm1 = np.concatenate([np.arange(0, HD, 2), np.arange(1, HD, 2)])
    permq = (np.arange(H)[:, None] * HD + perm1[None, :]).reshape(-1)
    permk = (np.arange(KV)[:, None] * HD + perm1[None, :]).reshape(-1)
    scale = np.float32(1.0 / np.sqrt(HD))
    wq_p = np.asarray(wq, np.float32)[permq] * scale
    wk_p = np.asarray(wk, np.float32)[permk]
    wv32 = np.asarray(wv, np.float32)
    wo32 = np.asarray(wo, np.float32)
    cosT = np.asarray(freqs_cos, np.float32).T
    sinT = np.asarray(freqs_sin, np.float32).T
    ccb = np.ascontiguousarray(np.concatenate([cosT, cosT], 0))
    ssb = np.ascontiguousarray(np.concatenate([sinT, -sinT], 0))
    tp = np.arange(128, dtype=np.int64)[:, None]
    sf = np.arange(SB, dtype=np.int64)[None, :]
    masks = np.stack(
        [(sf >= tp + 128 * m).astype(NPBF) for m in range(HPC)], 0)
    ones = np.ones((128, 1), NPBF)
    ident = np.eye(128, dtype=NPBF)

    in_maps = []
    for i in range(NCORES):
        wqkv = np.concatenate(
            [wq_p[i * EQ:(i + 1) * EQ],
             wk_p[i * HD:(i + 1) * HD],
             wv32[i * HD:(i + 1) * HD]], 0)
        wqkvt = np.ascontiguousarray(wqkv.T).astype(NPBF)
        wot = np.ascontiguousarray(
            wo32[i * EQ:(i + 1) * EQ, :].T).astype(NPBF)
        in_maps.append(dict(xt=xT, wqkvt=wqkvt, wot=wot, cc=ccb, ss=ssb,
                            masks=masks, ones=ones, ident=ident))
    return in_maps


def _run(in_maps, trace=False):
    nc = _get_program()
    return run_bass_kernel_spmd(
        nc, in_maps, core_ids=list(range(NCORES)), trace=trace)


def _assemble(res):
    shards = [np.asarray(res.results[i]["out"]) for i in range(NCORES)]
    return np.concatenate(shards, axis=1).reshape(B, S, D).astype(np.float32)


def kernel(x, freqs_cos, freqs_sin, wq, wk, wv, wo):
    in_maps = _host_prep(x, freqs_cos, freqs_sin, wq, wk, wv, wo)
    res = _run(in_maps, trace=False)
    return _assemble(res)


def _build_sharded():
    """Mirror of bass2jax.run_bass_via_pjrt's multi-core path, split so the
    jitted callable and device-resident inputs can be reused for timing."""
    import jax
    from jax.experimental.shard_map import shard_map
    from jax.sharding import Mesh, PartitionSpec

    import concourse.mybir as mb
    from concourse import bass2jax

    nc = _get_program()
    bass2jax.install_neuronx_cc_hook()
    part_name = (nc.partition_id_tensor.name
                 if nc.partition_id_tensor else None)
    in_names, out_names, out_avals, zero_outs = [], [], [], []
    for alloc in nc.m.functions[0].allocations:
        if not isinstance(alloc, mb.MemoryLocationSet):
            continue
        name = alloc.memorylocations[0].name
        if alloc.kind == "ExternalInput":
            if name != part_name:
                in_names.append(name)
        elif alloc.kind == "ExternalOutput":
            out_names.append(name)
            shape = tuple(alloc.tensor_shape)
            dtype = mb.dt.np(alloc.dtype)
            out_avals.append(jax.core.ShapedArray(shape, dtype))
            zero_outs.append(np.zeros(shape, dtype))
    n_params = len(in_names)
    all_names = in_names + out_names
    if part_name is not None:
        all_names = all_names + [part_name]

    def _body(*args):
        operands = list(args)
        if part_name is not None:
            operands.append(bass2jax.partition_id_tensor())
        outs = bass2jax._bass_exec_p.bind(
            *operands,
            out_avals=tuple(out_avals),
            in_names=tuple(all_names),
            out_names=tuple(out_names),
            lowering_input_output_aliases=(),
            sim_require_finite=True,
            sim_require_nnan=True,
            nc=nc,
        )
        return tuple(outs)

    devices = jax.devices()[:NCORES]
    mesh = Mesh(np.asarray(devices), ("core",))
    n_outs = len(out_names)
    sharded = jax.jit(
        shard_map(
            _body, mesh=mesh,
            in_specs=(PartitionSpec("core"),) * (n_params + n_outs),
            out_specs=(PartitionSpec("core"),) * n_outs,
            check_rep=False,
        ),
        donate_argnums=tuple(range(n_params, n_params + n_outs)),
        keep_unused=True,
    )
    return sharded, in_names, out_names, out_avals, zero_outs, mesh


def kernel_profiled(x, freqs_cos, freqs_sin, wq, wk, wv, wo, iters=12):
    """Returns (output, per-execution wall ns). Times repeated on-device
    executions with inputs pre-placed on the devices."""
    import time

    import jax
    from jax.sharding import NamedSharding, PartitionSpec

    in_maps = _host_prep(x, freqs_cos, freqs_sin, wq, wk, wv, wo)
    sharded, in_names, out_names, out_avals, zero_outs, mesh = _build_sharded()
    spec = NamedSharding(mesh, PartitionSpec("core"))
    concat_in = [
        jax.device_put(
            np.concatenate([in_maps[c][n] for c in range(NCORES)], axis=0),
            spec)
        for n in in_names
    ]

    def zeros():
        return [
            jax.device_put(
                np.zeros((NCORES * z.shape[0], *z.shape[1:]), z.dtype), spec)
            for z in zero_outs
        ]

    out_arrs = sharded(*concat_in, *zeros())  # warmup & result
    jax.block_until_ready(out_arrs)
    result = [np.asarray(a) for a in out_arrs]

    zsets = [zeros() for _ in range(iters)]
    jax.block_until_ready(zsets)
    t0 = time.perf_counter()
    last = None
    for zs in zsets:
        last = sharded(*concat_in, *zs)
    jax.block_until_ready(last)
    t1 = time.perf_counter()
    per_iter_ns = (t1 - t0) / iters * 1e9

    res_maps = [
        {n: result[i].reshape(NCORES, *out_avals[i].shape)[c]
         for i, n in enumerate(out_names)}
        for c in range(NCORES)
    ]

    class _R:
        results = res_maps

    return _assemble(_R), per_iter_ns
